# revision 33
# baseline (speedup 1.0000x reference)
"""DYAN encoder (FISTA sparse coding) as a Bass/Tile kernel on 8 trn2 NeuronCores.

Algorithm notes
---------------
reference computes, with D [T=10, K=645] (normalized dictionary), Y = x[0] [10, P]:
    A   = I - D^T D / L,  c = D^T Y / L,  lam = 0.1 / L
    y_0 = x_0 = 0
    for j in 0..99:   (the early-stop never triggers for this data)
        w      = A y_j + c = y_j + (1/L) D^T (Y - D y_j)
        x_{j+1} = softshrink(w, lam)
        y_{j+1} = (1+tt_j) x_{j+1} - tt_j x_j
Since A is I minus a rank-10 term, each iteration only needs thin matmuls:
    u_j = Y - D x_j                    [10, P]   (PE, contraction 645)
    r_j = (1+tt) u_j - tt u_{j-1}      (momentum folded into the residual)
    w   = (1/L) D^T r_j - tt x_{j-1} + (1+tt) x_j + ... (identity parts)
    x_{j+1} = shrink((1/L) D^T r_j - tt x_{j-1}  +  (1+tt) x_j)
The (1/L)(1+tt) / -(1/L)tt scalings ride the PSUM->SBUF copies of u (ScalarE),
the -tt x_{j-1} term is a scaled-identity matmul on PE, and the (1+tt) x_j add
plus softshrink is one fused custom DVE op per chunk.

Sharding: pure data parallel over the pixel dim P (8192 -> 8 x 1024).
"""

import os
import numpy as np

T = 10
NDICT = 161
K = 4 * NDICT + 1          # 645
P_FULL = 8192
N_CORES = 8
P = P_FULL // N_CORES      # 1024
NH = 512                   # psum-bank half width (fp32)
CH = [128, 128, 128, 128, 128, 5]   # K split into partition chunks
OFF = [0, 128, 256, 384, 512, 640]
NITER = 100
LAMBD = 0.1

# debug/ablation flags
ACT_COPY = os.environ.get("FISTA_ACT_COPY", "1") == "1"  # A/B copies on ScalarE (else VectorE)

_cache = {}


# --------------------------------------------------------------------------- #
# custom DVE ops
# --------------------------------------------------------------------------- #
def _register_dve_op(name, spec):
    import concourse.dve_ops as dve_ops_mod
    from concourse.dve_spec import lower, _has_src1
    from concourse.dve_uop import DveOpSpec

    for o in dve_ops_mod.OPS:
        if o.name == name:
            return o
    row = dve_ops_mod._CUSTOM_DVE_ROW_BASE + len(dve_ops_mod.OPS)
    assert row < 0x20, "DVE opcode rows exhausted"
    shas = {}
    for ver in ("v3", "v4"):
        s = DveOpSpec(name=name, opcode=row, uops=lower(spec, ver=ver),
                      rd1_en=_has_src1(spec))
        shas[ver] = s.sha(ver)
    op = dve_ops_mod.DveOp(name, spec, subdim=False, uops_sha=shas)
    dve_ops_mod.OPS.append(op)
    dve_ops_mod._SUB_OPCODE_FOR_NAME[name] = row
    dve_ops_mod.CUSTOM_DVE_SPECS[name] = spec
    return op


def _get_shrink_op():
    """out = v - clamp(v, -s1, s1) with v = in0 + s0*in1  (softshrink fused
    with the momentum-weighted x add; in0 comes straight from PSUM)."""
    from concourse.dve_spec import Spec, Src0, Src1, C0, C1, C2, maxx, minn

    v = Src0 + C0 * Src1
    body = v - minn(maxx(v, C2), C1)

    def _ref(in0, in1, s0, s1, imm2):
        v = in0.astype(np.float32) + np.float32(s0) * in1.astype(np.float32)
        return v - np.minimum(np.maximum(v, np.float32(imm2)), np.float32(s1))

    return _register_dve_op("FISTA_SHRINK_ANT", Spec(body=body, reference=_ref))


def _get_shrink0_op():
    """out = in0 - clamp(in0, -s1, s1)  (softshrink only; used at iteration 0
    where x_0 = 0 so there is no momentum term)."""
    from concourse.dve_spec import Spec, Src0, C0, C1, maxx, minn

    body = Src0 - minn(maxx(Src0, C0), C1)

    def _ref(in0, in1, s0, s1, imm2):
        v = in0.astype(np.float32)
        return v - np.minimum(np.maximum(v, np.float32(s0)), np.float32(s1))

    return _register_dve_op("FISTA_SHRINK0_ANT", Spec(body=body, reference=_ref))


# --------------------------------------------------------------------------- #
# host-side precompute
# --------------------------------------------------------------------------- #
def _build_dictionary(rr, theta, t):
    i = np.arange(t, dtype=np.float64)[:, None]
    rr = rr.astype(np.float64)
    theta = theta.astype(np.float64)
    rp = rr[None, :] ** i
    sgn = np.where(np.arange(t)[:, None] % 2 == 0, 1.0, -1.0)
    c = np.cos(i * theta[None, :])
    s = np.sin(i * theta[None, :])
    ones = np.ones((t, 1))
    dic = np.concatenate([ones, rp * c, sgn * rp * c, rp * s, sgn * rp * s], axis=1)
    g = np.linalg.norm(dic, axis=0)
    g = np.where(g == 0, np.sqrt(t), g)
    return dic / g


def _momentum_coeffs(n_iter):
    ts = []
    t = 1.0
    for _ in range(n_iter):
        t_new = (1.0 + np.sqrt(1.0 + 4.0 * t * t)) / 2.0
        ts.append((t - 1.0) / t_new)
        t = t_new
    return np.asarray(ts, dtype=np.float32)


# --------------------------------------------------------------------------- #
# device module
# --------------------------------------------------------------------------- #
def _build_module(lam, linv, tts):
    import concourse.bacc as bacc
    import concourse.mybir as mybir
    import concourse.tile as tile

    F32 = mybir.dt.float32
    F32R = mybir.dt.float32r
    shrink_op = _get_shrink_op()
    shrink0_op = _get_shrink0_op()

    nc = bacc.Bacc("TRN2", target_bir_lowering=False, debug=False)

    # All matmul stationaries are zero-padded to the full 128 contraction
    # rows: the PE HAM activity monitor counts active array ROWS, and any
    # sustained stream of partial-row matmuls (42-row W, 10-row sy) gets
    # duty-cycled to K=4/8 (1.2 GHz). Padding rows is free: matmul time is
    # set by the moving free dim, and zero weights kill the junk lanes.
    y_d = nc.dram_tensor("y_in", [T, P], F32R, kind="ExternalInput").ap()
    sy_d = nc.dram_tensor("s_y", [128, 42], F32R, kind="ExternalInput").ap()
    sd_d = nc.dram_tensor("s_d", [768, 42], F32R, kind="ExternalInput").ap()
    wab_d = nc.dram_tensor("w_ab", [128, 768], F32R, kind="ExternalInput").ap()
    i_d = nc.dram_tensor("i_const", [128, 128], F32R, kind="ExternalInput").ap()
    out_d = nc.dram_tensor("out", [K, P], F32, kind="ExternalOutput").ap()

    # per-iteration scalars (fp32-exact python floats)
    tt_prev = [0.0] + [float(tts[j]) for j in range(NITER - 1)]
    lam_f = float(np.float32(lam))
    linv_f = float(np.float32(linv))

    with tile.TileContext(nc) as tc:
        with (
            tc.tile_pool(name="const", bufs=1) as const,
            tc.tile_pool(name="state", bufs=1) as state,
            tc.tile_pool(name="iscp", bufs=3) as iscp,
            tc.tile_pool(name="upool", bufs=1, space="PSUM") as upool,
            tc.tile_pool(name="wpool", bufs=3, space="PSUM") as wpool,
        ):
            sy_t = const.tile([128, 42], F32R, tag="sy", name="sy_t")
            wab_t = const.tile([128, 768], F32R, tag="wab", name="wab_t")
            i_t = const.tile([128, 128], F32R, tag="ic", name="i_t")
            sd_t = [const.tile([128, 42], F32R, tag=f"sd{c}", name=f"sd_t{c}") for c in range(5)]

            nc.sync.dma_start(out=sy_t[:], in_=sy_d[:])

            # yg: the Y tile doubles as carrier of the 5-row tail x-chunk
            # (rows 10:15, selected by s_y rows 10:15 = -D_tail in the
            # u-matmul and by isc[:, 64:69] in the identity matmul). Rows
            # 15:128 are zero padding. Three generations like xt.
            yg = [state.tile([128, P], F32R, tag=f"yg{g}", name=f"yg{g}")
                  for g in range(3)]
            xt = [[state.tile([128, P], F32R, tag=f"x{g}_{c}", name=f"x{g}_{c}") for c in range(5)]
                  for g in range(3)]
            ab_ts = [state.tile([128, P], F32R, tag=f"AB{p}", name=f"ab_t{p}")
                     for p in range(2)]
            # dead/padded contraction lanes must be finite (zeros): ab rows
            # 10:128 (dead lanes + unwritten B slot at j=0 + row padding) and
            # yg rows 10:128 (tail x_0 = 0 + padding). GpSimd memset — the
            # engine is otherwise idle and this keeps the DMA ring short.
            for p in range(2):
                nc.gpsimd.memset(ab_ts[p][:].bitcast(F32), 0.0)
            for g in range(3):
                nc.gpsimd.memset(yg[g][:].bitcast(F32), 0.0)
                nc.sync.dma_start(out=yg[g][5:5 + T, :], in_=y_d[:])
            # bulk weights after the iteration-0 critical tiles (sy, yg)
            nc.sync.dma_start(out=wab_t[:], in_=wab_d[:])
            nc.sync.dma_start(out=i_t[:], in_=i_d[:])
            for c in range(5):
                nc.sync.dma_start(out=sd_t[c][:], in_=sd_d[OFF[c]:OFF[c] + 128, :])

            # Iteration specialization (avoids any zero-init):
            #   j=0: x_0 = x_{-1} = 0 -> u_0 = Y (no x-stream), no identity
            #        matmul, no m2old, plain shrink (no momentum add).
            #   j=1: tt_prev = tts[0] = 0 -> no identity matmul; m2old runs
            #        with B_0 (which is itself zero since b_scale(0) = 0).
            isc_next = None
            for j in range(NITER):
                ttp = tt_prev[j]
                gm1, g0, g1 = (j + 2) % 3, j % 3, (j + 1) % 3
                ab_cur = ab_ts[j % 2]
                ab_next = ab_ts[(j + 1) % 2]
                a_scale = float(np.float32((1.0 + ttp) * linv_f))
                b_scale = float(np.float32(-float(tts[j]) * linv_f))
                has_ux = j >= 1        # x_j nonzero
                has_id = ttp != 0.0    # j >= 2

                # scaled identity for iteration j+1's -tt * x_{j-1} term,
                # hoisted one iteration so ScalarE finishes it well before
                # the id-matmuls' LDWEIGHTS needs it (was a ~915ns PE stall
                # per iteration when computed just-in-time)
                isc = isc_next
                if j + 1 < NITER and tts[j] != 0.0:
                    isc_next = iscp.tile([128, 128], F32R, tag="isc",
                                         name="isc")
                    if ACT_COPY:
                        nc.scalar.mul(isc_next[:], i_t[:],
                                      float(np.float32(-float(tts[j]))))
                    else:
                        nc.vector.tensor_scalar_mul(
                            isc_next[:], i_t[:],
                            float(np.float32(-float(tts[j]))))

                # u = Y - D x_j, replicated at partition groups 0/32.
                # Per-half PSUM tiles (1 bank each) so the WAR against this
                # iteration's A/B copies clears per half, not per iteration —
                # a whole-tile WAR stalled the next u-matmul ~457ns once per
                # 2 iterations, and each stall cost a 27us HAM re-throttle.
                # tail momentum: ab_next rows 64:69 = -tts[j] * x_j[tail]
                # (x_j tail = yg[g0] rows 0:5, produced last iteration, so
                # this ScalarE copy is off the critical path)
                if j < NITER - 1:
                    nc.scalar.mul(ab_next[64:69, :], yg[g0][0:5, :],
                                  float(np.float32(-float(tts[j]))))
                for h in (0, 1):
                    sl = slice(NH * h, NH * (h + 1))
                    u_ps = upool.tile([42, NH], F32, tag=f"u{h}", name=f"u_ps{h}")
                    # consume chunks in shrink-production order: the tail
                    # (carried by yg) is shrunk FIRST each iteration, so the
                    # yg-matmul leads and the sd chunks follow as their
                    # shrinks land
                    nc.tensor.matmul(u_ps[:], sy_t[:], yg[g0][:, sl],
                                     start=True, stop=not has_ux)
                    if has_ux:
                        for c in range(5):
                            nc.tensor.matmul(u_ps[:], sd_t[c][:],
                                             xt[g0][c][:, sl],
                                             start=False, stop=(c == 4))
                    # scaled copies: A_j = (1+tt)/L u_j (used now),
                    #                B_j = -tts[j]/L u_j (used next iteration)
                    if ACT_COPY:
                        nc.scalar.mul(ab_cur[0:T, sl], u_ps[0:T, :], a_scale)
                        if j < NITER - 1:
                            nc.scalar.mul(ab_next[32:42, sl],
                                          u_ps[32:42, :], b_scale)
                    else:
                        nc.vector.tensor_scalar_mul(ab_cur[0:T, sl],
                                                    u_ps[0:T, :], a_scale)
                        if j < NITER - 1:
                            nc.vector.tensor_scalar_mul(ab_next[32:42, sl],
                                                        u_ps[32:42, :], b_scale)

                def x_ap(g, c):
                    # chunk c of x_g: chunks 0..4 are full xt tiles, the
                    # 5-row tail chunk lives at rows 10:15 of yg[g]
                    if c == 5:
                        return yg[g][0:5, :]
                    return xt[g][c][:]

                for wv in (0, 1):
                    # tail chunk leads: its shrink output (yg rows 0:5) gates
                    # the next iteration's first u-matmul, so produce it first
                    cs = [5, 0, 1] if wv == 0 else [2, 3, 4]
                    wt = {c: wpool.tile([CH[c], P], F32, tag="w", name=f"w{c}")
                          for c in cs}

                    def w_ap(c, sl=slice(None)):
                        return wt[c][:, sl]
                    # Chunk-major emission: finish all 4 matmuls of chunk c
                    # (id h0/h1 + W h0/h1) before starting chunk c+1, so the
                    # DVE shrink of chunk c can start 6 matmul-slots earlier
                    # than with the h-major sweep. Identity part first:
                    # w = -tt * x_{j-1} (tail chunk: isc cols 0:5 pick the
                    # tail x rows 0:5 of yg), then w += [D;0;D]^T [A;junk;B].
                    for c in cs:
                        for h in (0, 1):
                            sl = slice(NH * h, NH * (h + 1))
                            if has_id and c != 5:
                                nc.tensor.matmul(
                                    w_ap(c, sl), isc[:, 0:CH[c]],
                                    xt[gm1][c][:, sl],
                                    start=True, stop=False)
                        for h in (0, 1):
                            sl = slice(NH * h, NH * (h + 1))
                            nc.tensor.matmul(
                                w_ap(c, sl),
                                wab_t[:, 128 * c:128 * c + CH[c]],
                                ab_cur[:, sl],
                                start=(not has_id) or c == 5, stop=True)
                    # x_{j+1} = shrink(w + (1+tt) x_j). Chunk 4 is produced
                    # last each iteration and gates the next iteration's
                    # u-matmul completion -> A-copy -> W chain; splitting just
                    # its shrink per half releases each u-half ~560ns earlier
                    # for +65ns of DVE time.
                    for c in cs:
                        hs = ((slice(0, NH), slice(NH, P)) if c == 4
                              else (slice(0, P),))
                        for sl_s in hs:
                            if has_ux:
                                nc.vector._custom_dve(
                                    shrink_op, out=x_ap(g1, c)[:, sl_s],
                                    in0=w_ap(c)[:, sl_s],
                                    in1=x_ap(g0, c)[:, sl_s],
                                    s0=float(np.float32(1.0 + ttp)), s1=lam_f,
                                    imm2=-lam_f)
                            else:
                                nc.vector._custom_dve(
                                    shrink0_op, out=x_ap(g1, c)[:, sl_s],
                                    in0=w_ap(c)[:, sl_s],
                                    s0=-lam_f, s1=lam_f)
                        if j == NITER - 1:
                            nc.sync.dma_start(
                                out=out_d[OFF[c]:OFF[c] + CH[c], :],
                                in_=x_ap(g1, c).bitcast(F32))

    nc.compile()
    return nc


# --------------------------------------------------------------------------- #
# entry point
# --------------------------------------------------------------------------- #
def _prepare(x, Drr, Dtheta, t):
    x = np.asarray(x, dtype=np.float32)
    d64 = _build_dictionary(np.asarray(Drr), np.asarray(Dtheta), t)
    dtd = d64.T @ d64
    lspec = np.linalg.norm(dtd, ord=2)
    linv = 1.0 / lspec
    lam = LAMBD * linv
    d32 = d64.astype(np.float32)
    tts = _momentum_coeffs(NITER)

    # u = Y - D x is produced replicated at partition offsets 0 and 32 (the
    # 0-copy feeds the A scaled-copy, the 32-copy feeds the B scaled-copy).
    # Stationaries are row-padded with zeros to the full 128 contraction rows
    # (see _build_module) — padding rows multiply zeroed/junk moving lanes.
    # s_y rows 5:15 = +I (the Y pass); rows 0:5 = -D_tail (the 5-row tail
    # x-chunk rides rows 0:5 of the Y tile)
    s_y = np.zeros((128, 42), dtype=np.float32)
    for r in (0, 1):
        s_y[5 + np.arange(T), 32 * r + np.arange(T)] = 1.0
        s_y[0:5, 32 * r:32 * r + T] = -d32[:, OFF[5]:OFF[5] + CH[5]].T
    s_d = np.zeros((768, 42), dtype=np.float32)
    for r in (0, 1):
        s_d[0:OFF[5], 32 * r:32 * r + T] = -d32[:, 0:OFF[5]].T
    # merged rank-20 stationary: rows 0..9 multiply A, rows 32..41 multiply B.
    # Rows 64:69 of the tail block hold I5: the tail chunk's -tt*x_{j-1}
    # momentum rides the W-matmul via ab rows 64:69 (a scaled ScalarE copy of
    # the tail x), replacing two identity matmuls.
    w_ab = np.zeros((128, 768), dtype=np.float32)
    for c in range(6):
        w_ab[0:T, 128 * c:128 * c + CH[c]] = d32[:, OFF[c]:OFF[c] + CH[c]]
        w_ab[32:42, 128 * c:128 * c + CH[c]] = d32[:, OFF[c]:OFF[c] + CH[c]]
    w_ab[64:69, 128 * 5:128 * 5 + CH[5]] = np.eye(5, dtype=np.float32)
    i_const = np.eye(128, dtype=np.float32)
    return x, lam, linv, tts, s_y, s_d, w_ab, i_const


def run(x, Drr, Dtheta, T_in, trace=False):
    from concourse.bass_utils import run_bass_kernel_spmd

    t = int(np.asarray(T_in))
    assert t == T
    x, lam, linv, tts, s_y, s_d, w_ab, i_const = _prepare(x, Drr, Dtheta, t)

    key = ("mod", float(np.float32(lam)), float(np.float32(linv)))
    if key not in _cache:
        _cache[key] = _build_module(lam, linv, tts)
    nc = _cache[key]

    in_maps = []
    for core in range(N_CORES):
        in_maps.append({
            "y_in": np.ascontiguousarray(x[0, :, core * P:(core + 1) * P]),
            "s_y": s_y,
            "s_d": s_d,
            "w_ab": w_ab,
            "i_const": i_const,
        })
    res = run_bass_kernel_spmd(nc, in_maps, list(range(N_CORES)), trace=trace)
    out = np.concatenate([res.results[c]["out"] for c in range(N_CORES)], axis=1)
    return out[None, :, :].astype(np.float32), res


def kernel(x, Drr, Dtheta, T, **kw):
    out, _ = run(x, Drr, Dtheta, T, trace=bool(os.environ.get("FISTA_TRACE")))
    return out



# revision 34
# speedup vs baseline: 1.0522x; 1.0522x over previous
"""DYAN encoder (FISTA sparse coding) as a Bass/Tile kernel on 8 trn2 NeuronCores.

Algorithm notes
---------------
reference computes, with D [T=10, K=645] (normalized dictionary), Y = x[0] [10, P]:
    A   = I - D^T D / L,  c = D^T Y / L,  lam = 0.1 / L
    y_0 = x_0 = 0
    for j in 0..99:   (the early-stop never triggers for this data)
        w      = A y_j + c = y_j + (1/L) D^T (Y - D y_j)
        x_{j+1} = softshrink(w, lam)
        y_{j+1} = (1+tt_j) x_{j+1} - tt_j x_j
Since A is I minus a rank-10 term, each iteration only needs thin matmuls:
    u_j = Y - D x_j                    [10, P]   (PE, contraction 645)
    r_j = (1+tt) u_j - tt u_{j-1}      (momentum folded into the residual)
    w   = (1/L) D^T r_j - tt x_{j-1} + (1+tt) x_j + ... (identity parts)
    x_{j+1} = shrink((1/L) D^T r_j - tt x_{j-1}  +  (1+tt) x_j)
The (1/L)(1+tt) / -(1/L)tt scalings ride the PSUM->SBUF copies of u (ScalarE),
the -tt x_{j-1} term is a scaled-identity matmul on PE, and the (1+tt) x_j add
plus softshrink is one fused custom DVE op per chunk.

Sharding: pure data parallel over the pixel dim P (8192 -> 8 x 1024).
"""

import os
import numpy as np

T = 10
NDICT = 161
K = 4 * NDICT + 1          # 645
P_FULL = 8192
N_CORES = 8
P = P_FULL // N_CORES      # 1024
NH = 512                   # psum-bank half width (fp32)
CH = [128, 128, 128, 128, 128, 5]   # K split into partition chunks
OFF = [0, 128, 256, 384, 512, 640]
NITER = 100
LAMBD = 0.1

# debug/ablation flags
ACT_COPY = os.environ.get("FISTA_ACT_COPY", "1") == "1"  # A/B copies on ScalarE (else VectorE)

_cache = {}


# --------------------------------------------------------------------------- #
# custom DVE ops
# --------------------------------------------------------------------------- #
def _register_dve_op(name, spec):
    import concourse.dve_ops as dve_ops_mod
    from concourse.dve_spec import lower, _has_src1
    from concourse.dve_uop import DveOpSpec

    for o in dve_ops_mod.OPS:
        if o.name == name:
            return o
    row = dve_ops_mod._CUSTOM_DVE_ROW_BASE + len(dve_ops_mod.OPS)
    assert row < 0x20, "DVE opcode rows exhausted"
    shas = {}
    for ver in ("v3", "v4"):
        s = DveOpSpec(name=name, opcode=row, uops=lower(spec, ver=ver),
                      rd1_en=_has_src1(spec))
        shas[ver] = s.sha(ver)
    op = dve_ops_mod.DveOp(name, spec, subdim=False, uops_sha=shas)
    dve_ops_mod.OPS.append(op)
    dve_ops_mod._SUB_OPCODE_FOR_NAME[name] = row
    dve_ops_mod.CUSTOM_DVE_SPECS[name] = spec
    return op


def _get_shrink_op():
    """out = v - clamp(v, -s1, s1) with v = in0 + s0*in1  (softshrink fused
    with the momentum-weighted x add; in0 comes straight from PSUM)."""
    from concourse.dve_spec import Spec, Src0, Src1, C0, C1, C2, maxx, minn

    v = Src0 + C0 * Src1
    body = v - minn(maxx(v, C2), C1)

    def _ref(in0, in1, s0, s1, imm2):
        v = in0.astype(np.float32) + np.float32(s0) * in1.astype(np.float32)
        return v - np.minimum(np.maximum(v, np.float32(imm2)), np.float32(s1))

    return _register_dve_op("FISTA_SHRINK_ANT", Spec(body=body, reference=_ref))


def _get_shrink0_op():
    """out = in0 - clamp(in0, -s1, s1)  (softshrink only; used at iteration 0
    where x_0 = 0 so there is no momentum term)."""
    from concourse.dve_spec import Spec, Src0, C0, C1, maxx, minn

    body = Src0 - minn(maxx(Src0, C0), C1)

    def _ref(in0, in1, s0, s1, imm2):
        v = in0.astype(np.float32)
        return v - np.minimum(np.maximum(v, np.float32(s0)), np.float32(s1))

    return _register_dve_op("FISTA_SHRINK0_ANT", Spec(body=body, reference=_ref))


# --------------------------------------------------------------------------- #
# host-side precompute
# --------------------------------------------------------------------------- #
def _build_dictionary(rr, theta, t):
    i = np.arange(t, dtype=np.float64)[:, None]
    rr = rr.astype(np.float64)
    theta = theta.astype(np.float64)
    rp = rr[None, :] ** i
    sgn = np.where(np.arange(t)[:, None] % 2 == 0, 1.0, -1.0)
    c = np.cos(i * theta[None, :])
    s = np.sin(i * theta[None, :])
    ones = np.ones((t, 1))
    dic = np.concatenate([ones, rp * c, sgn * rp * c, rp * s, sgn * rp * s], axis=1)
    g = np.linalg.norm(dic, axis=0)
    g = np.where(g == 0, np.sqrt(t), g)
    return dic / g


def _momentum_coeffs(n_iter):
    ts = []
    t = 1.0
    for _ in range(n_iter):
        t_new = (1.0 + np.sqrt(1.0 + 4.0 * t * t)) / 2.0
        ts.append((t - 1.0) / t_new)
        t = t_new
    return np.asarray(ts, dtype=np.float32)


# --------------------------------------------------------------------------- #
# device module
# --------------------------------------------------------------------------- #
def _build_module(lam, linv, tts):
    import concourse.bacc as bacc
    import concourse.mybir as mybir
    import concourse.tile as tile

    F32 = mybir.dt.float32
    F32R = mybir.dt.float32r
    shrink_op = _get_shrink_op()
    shrink0_op = _get_shrink0_op()

    nc = bacc.Bacc("TRN2", target_bir_lowering=False, debug=False)

    # All matmul stationaries are zero-padded to the full 128 contraction
    # rows: the PE HAM activity monitor counts active array ROWS, and any
    # sustained stream of partial-row matmuls (42-row W, 10-row sy) gets
    # duty-cycled to K=4/8 (1.2 GHz). Padding rows is free: matmul time is
    # set by the moving free dim, and zero weights kill the junk lanes.
    y_d = nc.dram_tensor("y_in", [T, P], F32R, kind="ExternalInput").ap()
    sy_d = nc.dram_tensor("s_y", [128, 42], F32R, kind="ExternalInput").ap()
    sd_d = nc.dram_tensor("s_d", [768, 42], F32R, kind="ExternalInput").ap()
    wab_d = nc.dram_tensor("w_ab", [128, 768], F32R, kind="ExternalInput").ap()
    i_d = nc.dram_tensor("i_const", [128, 128], F32R, kind="ExternalInput").ap()
    out_d = nc.dram_tensor("out", [K, P], F32, kind="ExternalOutput").ap()

    # per-iteration scalars (fp32-exact python floats)
    tt_prev = [0.0] + [float(tts[j]) for j in range(NITER - 1)]
    lam_f = float(np.float32(lam))
    linv_f = float(np.float32(linv))

    with tile.TileContext(nc) as tc:
        with (
            tc.tile_pool(name="const", bufs=1) as const,
            tc.tile_pool(name="state", bufs=1) as state,
            tc.tile_pool(name="iscp", bufs=3) as iscp,
            tc.tile_pool(name="upool", bufs=1, space="PSUM") as upool,
            tc.tile_pool(name="wpool", bufs=3, space="PSUM") as wpool,
        ):
            sy_t = const.tile([128, 42], F32R, tag="sy", name="sy_t")
            wab_t = const.tile([128, 768], F32R, tag="wab", name="wab_t")
            i_t = const.tile([128, 128], F32R, tag="ic", name="i_t")
            sd_t = [const.tile([128, 42], F32R, tag=f"sd{c}", name=f"sd_t{c}") for c in range(5)]

            nc.sync.dma_start(out=sy_t[:], in_=sy_d[:])

            # yg: the Y tile doubles as carrier of the 5-row tail x-chunk
            # (rows 10:15, selected by s_y rows 10:15 = -D_tail in the
            # u-matmul and by isc[:, 64:69] in the identity matmul). Rows
            # 15:128 are zero padding. Three generations like xt.
            yg = [state.tile([128, P], F32R, tag=f"yg{g}", name=f"yg{g}")
                  for g in range(3)]
            xt = [[state.tile([128, P], F32R, tag=f"x{g}_{c}", name=f"x{g}_{c}") for c in range(5)]
                  for g in range(3)]
            ab_ts = [state.tile([128, P], F32R, tag=f"AB{p}", name=f"ab_t{p}")
                     for p in range(2)]
            # dead/padded contraction lanes must be finite (zeros): ab rows
            # 10:128 (dead lanes + unwritten B slot at j=0 + row padding) and
            # yg rows 10:128 (tail x_0 = 0 + padding). GpSimd memset — the
            # engine is otherwise idle and this keeps the DMA ring short.
            for p in range(2):
                nc.gpsimd.memset(ab_ts[p][:].bitcast(F32), 0.0)
            for g in range(3):
                nc.gpsimd.memset(yg[g][:].bitcast(F32), 0.0)
                nc.sync.dma_start(out=yg[g][5:5 + T, :], in_=y_d[:])
            # bulk weights after the iteration-0 critical tiles (sy, yg)
            nc.sync.dma_start(out=wab_t[:], in_=wab_d[:])
            nc.sync.dma_start(out=i_t[:], in_=i_d[:])
            for c in range(5):
                nc.sync.dma_start(out=sd_t[c][:], in_=sd_d[OFF[c]:OFF[c] + 128, :])

            # Iteration specialization (avoids any zero-init):
            #   j=0: x_0 = x_{-1} = 0 -> u_0 = Y (no x-stream), no identity
            #        matmul, no m2old, plain shrink (no momentum add).
            #   j=1: tt_prev = tts[0] = 0 -> no identity matmul; m2old runs
            #        with B_0 (which is itself zero since b_scale(0) = 0).
            isc_next = None
            for j in range(NITER):
                ttp = tt_prev[j]
                gm1, g0, g1 = (j + 2) % 3, j % 3, (j + 1) % 3
                ab_cur = ab_ts[j % 2]
                ab_next = ab_ts[(j + 1) % 2]
                a_scale = float(np.float32((1.0 + ttp) * linv_f))
                b_scale = float(np.float32(-float(tts[j]) * linv_f))
                has_ux = j >= 1        # x_j nonzero
                has_id = ttp != 0.0    # j >= 2

                # scaled identity for iteration j+1's -tt * x_{j-1} term,
                # hoisted one iteration so ScalarE finishes it well before
                # the id-matmuls' LDWEIGHTS needs it (was a ~915ns PE stall
                # per iteration when computed just-in-time)
                isc = isc_next
                if j + 1 < NITER and tts[j] != 0.0:
                    isc_next = iscp.tile([128, 128], F32R, tag="isc",
                                         name="isc")
                    if ACT_COPY:
                        nc.scalar.mul(isc_next[:], i_t[:],
                                      float(np.float32(-float(tts[j]))))
                    else:
                        nc.vector.tensor_scalar_mul(
                            isc_next[:], i_t[:],
                            float(np.float32(-float(tts[j]))))

                # u = Y - D x_j, replicated at partition groups 0/32.
                # Per-half PSUM tiles (1 bank each) so the WAR against this
                # iteration's A/B copies clears per half, not per iteration —
                # a whole-tile WAR stalled the next u-matmul ~457ns once per
                # 2 iterations, and each stall cost a 27us HAM re-throttle.
                # tail momentum: ab_next rows 64:69 = -tts[j] * x_j[tail]
                # (x_j tail = yg[g0] rows 0:5, produced last iteration, so
                # this ScalarE copy is off the critical path)
                if j < NITER - 1:
                    nc.scalar.mul(ab_next[64:69, :], yg[g0][0:5, :],
                                  float(np.float32(-float(tts[j]))))
                for h in (0, 1):
                    sl = slice(NH * h, NH * (h + 1))
                    u_ps = upool.tile([42, NH], F32, tag=f"u{h}", name=f"u_ps{h}")
                    # consume chunks in shrink-production order: the tail
                    # (carried by yg) is shrunk FIRST each iteration, so the
                    # yg-matmul leads and the sd chunks follow as their
                    # shrinks land
                    nc.tensor.matmul(u_ps[:], sy_t[:], yg[g0][:, sl],
                                     start=True, stop=not has_ux)
                    if has_ux:
                        for c in range(5):
                            nc.tensor.matmul(u_ps[:], sd_t[c][:],
                                             xt[g0][c][:, sl],
                                             start=False, stop=(c == 4))
                    # scaled copies: A_j = (1+tt)/L u_j (used now),
                    #                B_j = -tts[j]/L u_j (used next iteration)
                    if ACT_COPY:
                        nc.scalar.mul(ab_cur[0:T, sl], u_ps[0:T, :], a_scale)
                        if j < NITER - 1:
                            nc.scalar.mul(ab_next[32:42, sl],
                                          u_ps[32:42, :], b_scale)
                    else:
                        nc.vector.tensor_scalar_mul(ab_cur[0:T, sl],
                                                    u_ps[0:T, :], a_scale)
                        if j < NITER - 1:
                            nc.vector.tensor_scalar_mul(ab_next[32:42, sl],
                                                        u_ps[32:42, :], b_scale)

                def x_ap(g, c):
                    # chunk c of x_g: chunks 0..4 are full xt tiles, the
                    # 5-row tail chunk lives at rows 10:15 of yg[g]
                    if c == 5:
                        return yg[g][0:5, :]
                    return xt[g][c][:]

                for wv in (0, 1):
                    # tail chunk leads: its shrink output (yg rows 0:5) gates
                    # the next iteration's first u-matmul, so produce it first
                    cs = [5, 0, 1] if wv == 0 else [2, 3, 4]
                    wt = {c: wpool.tile([CH[c], P], F32, tag="w", name=f"w{c}")
                          for c in cs}

                    def w_ap(c, sl=slice(None)):
                        return wt[c][:, sl]
                    # Chunk-major emission: finish all 4 matmuls of chunk c
                    # (id h0/h1 + W h0/h1) before starting chunk c+1, so the
                    # DVE shrink of chunk c can start 6 matmul-slots earlier
                    # than with the h-major sweep. Identity part first:
                    # w = -tt * x_{j-1} (tail chunk: isc cols 0:5 pick the
                    # tail x rows 0:5 of yg), then w += [D;0;D]^T [A;junk;B].
                    for c in cs:
                        for h in (0, 1):
                            sl = slice(NH * h, NH * (h + 1))
                            if has_id and c != 5:
                                nc.tensor.matmul(
                                    w_ap(c, sl), isc[:, 0:CH[c]],
                                    xt[gm1][c][:, sl],
                                    start=True, stop=False)
                        for h in (0, 1):
                            sl = slice(NH * h, NH * (h + 1))
                            nc.tensor.matmul(
                                w_ap(c, sl),
                                wab_t[:, 128 * c:128 * c + CH[c]],
                                ab_cur[:, sl],
                                start=(not has_id) or c == 5, stop=True)
                    # x_{j+1} = shrink(w + (1+tt) x_j)
                    for c in cs:
                        if has_ux:
                            nc.vector._custom_dve(
                                shrink_op, out=x_ap(g1, c),
                                in0=w_ap(c),
                                in1=x_ap(g0, c),
                                s0=float(np.float32(1.0 + ttp)), s1=lam_f,
                                imm2=-lam_f)
                        else:
                            nc.vector._custom_dve(
                                shrink0_op, out=x_ap(g1, c),
                                in0=w_ap(c),
                                s0=-lam_f, s1=lam_f)
                        if j == NITER - 1:
                            nc.sync.dma_start(
                                out=out_d[OFF[c]:OFF[c] + CH[c], :],
                                in_=x_ap(g1, c).bitcast(F32))

    nc.compile()
    return nc


# --------------------------------------------------------------------------- #
# entry point
# --------------------------------------------------------------------------- #
def _prepare(x, Drr, Dtheta, t):
    x = np.asarray(x, dtype=np.float32)
    d64 = _build_dictionary(np.asarray(Drr), np.asarray(Dtheta), t)
    dtd = d64.T @ d64
    lspec = np.linalg.norm(dtd, ord=2)
    linv = 1.0 / lspec
    lam = LAMBD * linv
    d32 = d64.astype(np.float32)
    tts = _momentum_coeffs(NITER)

    # u = Y - D x is produced replicated at partition offsets 0 and 32 (the
    # 0-copy feeds the A scaled-copy, the 32-copy feeds the B scaled-copy).
    # Stationaries are row-padded with zeros to the full 128 contraction rows
    # (see _build_module) — padding rows multiply zeroed/junk moving lanes.
    # s_y rows 5:15 = +I (the Y pass); rows 0:5 = -D_tail (the 5-row tail
    # x-chunk rides rows 0:5 of the Y tile)
    s_y = np.zeros((128, 42), dtype=np.float32)
    for r in (0, 1):
        s_y[5 + np.arange(T), 32 * r + np.arange(T)] = 1.0
        s_y[0:5, 32 * r:32 * r + T] = -d32[:, OFF[5]:OFF[5] + CH[5]].T
    s_d = np.zeros((768, 42), dtype=np.float32)
    for r in (0, 1):
        s_d[0:OFF[5], 32 * r:32 * r + T] = -d32[:, 0:OFF[5]].T
    # merged rank-20 stationary: rows 0..9 multiply A, rows 32..41 multiply B.
    # Rows 64:69 of the tail block hold I5: the tail chunk's -tt*x_{j-1}
    # momentum rides the W-matmul via ab rows 64:69 (a scaled ScalarE copy of
    # the tail x), replacing two identity matmuls.
    w_ab = np.zeros((128, 768), dtype=np.float32)
    for c in range(6):
        w_ab[0:T, 128 * c:128 * c + CH[c]] = d32[:, OFF[c]:OFF[c] + CH[c]]
        w_ab[32:42, 128 * c:128 * c + CH[c]] = d32[:, OFF[c]:OFF[c] + CH[c]]
    w_ab[64:69, 128 * 5:128 * 5 + CH[5]] = np.eye(5, dtype=np.float32)
    i_const = np.eye(128, dtype=np.float32)
    return x, lam, linv, tts, s_y, s_d, w_ab, i_const


def run(x, Drr, Dtheta, T_in, trace=False):
    from concourse.bass_utils import run_bass_kernel_spmd

    t = int(np.asarray(T_in))
    assert t == T
    x, lam, linv, tts, s_y, s_d, w_ab, i_const = _prepare(x, Drr, Dtheta, t)

    key = ("mod", float(np.float32(lam)), float(np.float32(linv)))
    if key not in _cache:
        _cache[key] = _build_module(lam, linv, tts)
    nc = _cache[key]

    in_maps = []
    for core in range(N_CORES):
        in_maps.append({
            "y_in": np.ascontiguousarray(x[0, :, core * P:(core + 1) * P]),
            "s_y": s_y,
            "s_d": s_d,
            "w_ab": w_ab,
            "i_const": i_const,
        })
    res = run_bass_kernel_spmd(nc, in_maps, list(range(N_CORES)), trace=trace)
    out = np.concatenate([res.results[c]["out"] for c in range(N_CORES)], axis=1)
    return out[None, :, :].astype(np.float32), res


def kernel(x, Drr, Dtheta, T, **kw):
    out, _ = run(x, Drr, Dtheta, T, trace=bool(os.environ.get("FISTA_TRACE")))
    return out

